# revision 15
# baseline (speedup 1.0000x reference)
"""Trainium2 Bass kernel for nn_LocalEncoder (masked GRU + attention pooling).

v2 strategy (data-parallel over batch: 8 cores x 512 rows, 2 chunks of 256):
- Feature-major layout [U partitions, batch free]; bf16 matmuls, fp32 PSUM.
- Scan: blocked x/state DMA (W=20 steps per transfer, ~35 DMAs total vs 1200),
  K-projections for step t+1 pre-issued during step t (keeps PE warm and off
  the critical path), b1h folded into the Rh matmul via a const-1 row in the
  h tile, per-chunk sigmoid over both gate banks in one ACT call, elementwise
  work split across DVE and Pool so the two batch chunks pipeline.
- Mask semantics: row 100 of xaug is (1-m) scaled by -40 into the z-gate so
  masked steps freeze h exactly like the reference's jnp.where.
- Attention: 2 timesteps per PSUM group, software-pipelined on PE
  (acc-mms trail by 2 blocks, Vr-mms by 1), sigmoid batched over 2 banks,
  last@A1 term applied for ALL t on device; host subtracts the closed-form
  correction for masked (trailing) timesteps.
"""
import sys
sys.path.insert(0, "/opt/trn_rl_repo")
from contextlib import ExitStack

import numpy as np
import ml_dtypes

import concourse.bass as bass
import concourse.bacc as bacc
import concourse.tile as tile
from concourse import mybir
from concourse import bass_utils

bf16 = ml_dtypes.bfloat16
AF = mybir.ActivationFunctionType
OP = mybir.AluOpType

B, T, E, U = 4096, 200, 100, 100
NCORES = 8
BC = 256          # chunk width
NCHUNK = 2
PERCORE = BC * NCHUNK   # 512
NB = PERCORE            # batch columns per core
W = 20                  # scan DMA block (steps per transfer)
NBLK = T // W
AW = 20                 # attention state-load block
NK = T // 2             # attention 2-step groups

_CACHE = {}


def _build():
    nc = bacc.Bacc()
    dt = mybir.dt
    xaug = nc.dram_tensor("xaug", [102, T, NB], dt.bfloat16, kind="ExternalInput")
    wKz = nc.dram_tensor("wKz", [102, U], dt.bfloat16, kind="ExternalInput")
    wKr = nc.dram_tensor("wKr", [102, U], dt.bfloat16, kind="ExternalInput")
    wKh = nc.dram_tensor("wKh", [102, U], dt.bfloat16, kind="ExternalInput")
    wRz = nc.dram_tensor("wRz", [101, U], dt.bfloat16, kind="ExternalInput")
    wRr = nc.dram_tensor("wRr", [101, U], dt.bfloat16, kind="ExternalInput")
    wRh = nc.dram_tensor("wRh", [101, U], dt.bfloat16, kind="ExternalInput")
    wA1 = nc.dram_tensor("wA1", [U, U], dt.bfloat16, kind="ExternalInput")
    wA2 = nc.dram_tensor("wA2", [U, U], dt.bfloat16, kind="ExternalInput")
    wVr = nc.dram_tensor("wVr", [U, U], dt.bfloat16, kind="ExternalInput")
    wI = nc.dram_tensor("wI", [U, U], dt.bfloat16, kind="ExternalInput")
    wOnes = nc.dram_tensor("wOnes", [1, W, NB], dt.bfloat16, kind="ExternalInput")
    outraw = nc.dram_tensor("outraw", [U, NB], dt.float32, kind="ExternalOutput")
    lastout = nc.dram_tensor("lastout", [U, NB], dt.float32, kind="ExternalOutput")

    with tile.TileContext(nc) as tc, ExitStack() as octx:
        singles = octx.enter_context(tc.tile_pool(name="singles", bufs=1))
        dram = octx.enter_context(tc.tile_pool(name="dram", bufs=1, space="DRAM"))

        def load_w(dram_w, p):
            t = singles.tile([128, U], mybir.dt.bfloat16, tag=dram_w.name)
            nc.sync.dma_start(out=t[0:p, :], in_=dram_w[:, :])
            return t
        Kz, Kr, Kh = load_w(wKz, 102), load_w(wKr, 102), load_w(wKh, 102)
        Rz, Rr, Rh = load_w(wRz, 101), load_w(wRr, 101), load_w(wRh, 101)
        A1b, A2b, Vrb, Ib = (load_w(wA1, U), load_w(wA2, U),
                             load_w(wVr, U), load_w(wI, U))

        hinit = singles.tile([128, NB], mybir.dt.bfloat16, tag="hinit")
        nc.vector.memset(hinit, 0.0)
        nc.sync.dma_start(out=hinit[100:101, :], in_=wOnes[:, 0, :])
        last_bf = singles.tile([U, NB], mybir.dt.bfloat16, tag="last_bf")
        qdup = singles.tile([U, NB], mybir.dt.bfloat16, tag="qdup")

        state = dram.tile([U, T, NB], mybir.dt.bfloat16)

        # ---------------- scan ----------------
        with ExitStack() as ctx:
            xbp = ctx.enter_context(tc.tile_pool(name="xbp", bufs=2))
            stp = ctx.enter_context(tc.tile_pool(name="stp", bufs=2))
            gp = ctx.enter_context(tc.tile_pool(name="gp", bufs=2))
            pzr = ctx.enter_context(tc.tile_pool(name="pzr", bufs=2, space="PSUM"))
            phx = ctx.enter_context(tc.tile_pool(name="phx", bufs=2, space="PSUM"))

            xtiles = {}

            def issue_x(b):
                if b < NBLK:
                    xt = xbp.tile([128, W, NB], mybir.dt.bfloat16, tag="xb")
                    nc.sync.dma_start(out=xt[0:102, :, :],
                                      in_=xaug[:, b * W:(b + 1) * W, :])
                    xtiles[b] = xt
            issue_x(0)
            issue_x(1)

            sttiles = {}
            zr_cur = hx_cur = None

            def emit_kmms(tn, zr_t, hx_t):
                xs = xtiles[tn // W][0:102, tn % W, :]
                nc.tensor.matmul(zr_t[0:U, 0, :], lhsT=Kz[0:102, :], rhs=xs,
                                 start=True, stop=False, skip_group_check=True)
                nc.tensor.matmul(zr_t[0:U, 1, :], lhsT=Kr[0:102, :], rhs=xs,
                                 start=True, stop=False, skip_group_check=True)
                nc.tensor.matmul(hx_t[0:U, 0, :], lhsT=Kh[0:102, :], rhs=xs,
                                 start=True, stop=True, skip_group_check=True)

            for t in range(T):
                b, w = t // W, t % W
                if w == 0:
                    st_cur = stp.tile([128, W, NB], mybir.dt.bfloat16, tag="st")
                    sttiles[b] = st_cur
                    if b < 2:
                        nc.sync.dma_start(out=st_cur[100:101, :, :],
                                          in_=wOnes[:, :, :])
                    if b >= 1:
                        issue_x(b + 1)
                        nc.sync.dma_start(
                            out=state[:, (b - 1) * W:b * W, :],
                            in_=sttiles[b - 1][0:U, :, :])

                if t == 0:
                    zr_cur = pzr.tile([128, 2, NB], mybir.dt.float32, tag="zr")
                    hx_cur = phx.tile([128, 2, NB], mybir.dt.float32, tag="hx")
                    emit_kmms(0, zr_cur, hx_cur)

                if t == 0:
                    hp_full = hinit
                else:
                    hp_full = sttiles[(t - 1) // W][:, (t - 1) % W, :]

                # prefetch K-projections for t+1 into the other PSUM set
                if t + 1 < T:
                    zr_nxt = pzr.tile([128, 2, NB], mybir.dt.float32, tag="zr")
                    hx_nxt = phx.tile([128, 2, NB], mybir.dt.float32, tag="hx")
                    emit_kmms(t + 1, zr_nxt, hx_nxt)

                # recurrent matmuls, per chunk (keeps the two chains independent)
                for c in range(NCHUNK):
                    cs = slice(c * BC, (c + 1) * BC)
                    nc.tensor.matmul(zr_cur[0:U, 0, cs], lhsT=Rz[0:101, :],
                                     rhs=hp_full[0:101, cs],
                                     start=False, stop=True, skip_group_check=True)
                    nc.tensor.matmul(zr_cur[0:U, 1, cs], lhsT=Rr[0:101, :],
                                     rhs=hp_full[0:101, cs],
                                     start=False, stop=True, skip_group_check=True)
                    nc.tensor.matmul(hx_cur[0:U, 1, cs], lhsT=Rh[0:101, :],
                                     rhs=hp_full[0:101, cs],
                                     start=True, stop=True, skip_group_check=True)

                zrs = [None, None]
                for c in range(NCHUNK):
                    cs = slice(c * BC, (c + 1) * BC)
                    zz = gp.tile([U, 2, BC], mybir.dt.bfloat16, tag=f"zrs{c}")
                    nc.scalar.activation(zz, zr_cur[0:U, :, cs], AF.Sigmoid)
                    zrs[c] = zz

                ss = [None, None]
                for c in range(NCHUNK):
                    cs = slice(c * BC, (c + 1) * BC)
                    t1 = gp.tile([U, BC], mybir.dt.bfloat16, tag=f"t1{c}")
                    nc.vector.tensor_tensor(t1, zrs[c][:, 1, :],
                                            hx_cur[0:U, 1, cs], OP.mult)
                    s = gp.tile([U, BC], mybir.dt.bfloat16, tag=f"s{c}")
                    nc.vector.tensor_tensor(s, hx_cur[0:U, 0, cs], t1, OP.add)
                    ss[c] = s

                us = [None, None]
                for c in range(NCHUNK):
                    u = gp.tile([U, BC], mybir.dt.bfloat16, tag=f"u{c}")
                    nc.scalar.activation(u, ss[c], AF.Tanh)
                    us[c] = u

                ds = [None, None]
                for c in range(NCHUNK):
                    cs = slice(c * BC, (c + 1) * BC)
                    d = gp.tile([U, BC], mybir.dt.bfloat16, tag=f"d{c}")
                    nc.gpsimd.tensor_tensor(d, us[c], hp_full[0:U, cs], OP.subtract)
                    ds[c] = d
                for c in range(NCHUNK):
                    cs = slice(c * BC, (c + 1) * BC)
                    e = gp.tile([U, BC], mybir.dt.bfloat16, tag=f"e{c}")
                    nc.vector.tensor_tensor(e, zrs[c][:, 0, :], ds[c], OP.mult)
                    eng = nc.vector if c == 0 else nc.gpsimd
                    eng.tensor_tensor(st_cur[0:U, w, cs],
                                      hp_full[0:U, cs], e, OP.add)

                if t + 1 < T:
                    zr_cur, hx_cur = zr_nxt, hx_nxt

            # final block store + keep last state
            nc.sync.dma_start(out=state[:, (NBLK - 1) * W:T, :],
                              in_=sttiles[NBLK - 1][0:U, :, :])
            nc.vector.tensor_copy(last_bf, sttiles[NBLK - 1][0:U, W - 1, :])
            lo = singles.tile([U, NB], mybir.dt.float32, tag="lasto")
            nc.vector.tensor_copy(lo, sttiles[NBLK - 1][0:U, W - 1, :])
            nc.sync.dma_start(out=lastout[:, :], in_=lo)

        # ---------------- attention ----------------
        with ExitStack() as ctx:
            sp = ctx.enter_context(tc.tile_pool(name="sp", bufs=2))
            gp2 = ctx.enter_context(tc.tile_pool(name="gp2", bufs=2))
            psb = ctx.enter_context(tc.tile_pool(name="psb", bufs=2, space="PSUM"))
            pal = ctx.enter_context(tc.tile_pool(name="pal", bufs=1, space="PSUM"))
            pacc = ctx.enter_context(tc.tile_pool(name="pacc", bufs=1, space="PSUM"))
            pq = ctx.enter_context(tc.tile_pool(name="pq", bufs=1, space="PSUM"))

            # qdup = last @ A1  (broadcast over both chunks)
            qp = pq.tile([128, NB], mybir.dt.float32, tag="qp")
            nc.tensor.matmul(qp[0:U, :], lhsT=A1b[0:U, :], rhs=last_bf,
                             start=True, stop=True)
            nc.vector.tensor_copy(qdup, qp[0:U, :])

            atiles = {}

            def issue_att(ab):
                if ab * AW < T:
                    at = sp.tile([128, AW, NB], mybir.dt.bfloat16, tag="ab")
                    nc.sync.dma_start(out=at[0:U, :, :],
                                      in_=state[:, ab * AW:(ab + 1) * AW, :])
                    atiles[ab] = at
            issue_att(0)
            issue_att(1)

            def st_ap(tn):
                return atiles[tn // AW][0:U, tn % AW, :]

            acc = pacc.tile([128, NB], mybir.dt.float32, tag="acc")
            al = pal.tile([128, 2, NB], mybir.dt.float32, tag="al")
            gs = {}    # k -> sigmoid output
            tmps = {}  # k -> (tmp0, tmp1)

            def emit_acc(k, start, stop):
                t0, t1_ = tmps.pop(k)
                nc.tensor.matmul(acc[0:U, :], lhsT=Ib[0:U, :], rhs=t0,
                                 start=start, stop=False, skip_group_check=True)
                nc.tensor.matmul(acc[0:U, :], lhsT=Ib[0:U, :], rhs=t1_,
                                 start=False, stop=stop, skip_group_check=True)

            def emit_vr(k):
                for i in range(2):
                    nc.tensor.matmul(al[0:U, i, :], lhsT=Vrb[0:U, :],
                                     rhs=gs[k][:, i, :], start=True, stop=True)

            def emit_tmp(k):
                t0 = gp2.tile([U, NB], mybir.dt.bfloat16, tag="tmp0", name="tmp0")
                t1_ = gp2.tile([U, NB], mybir.dt.bfloat16, tag="tmp1", name="tmp1")
                nc.vector.tensor_tensor(t0, al[0:U, 0, :], st_ap(2 * k), OP.mult)
                nc.vector.tensor_tensor(t1_, al[0:U, 1, :], st_ap(2 * k + 1), OP.mult)
                tmps[k] = (t0, t1_)
                del gs[k]

            for k in range(NK):
                t0 = 2 * k
                ab = t0 // AW
                if t0 % AW == 2 and ab >= 1:
                    issue_att(ab + 1)

                if k >= 2:
                    emit_acc(k - 2, start=(k == 2), stop=False)
                sb = psb.tile([128, 2, NB], mybir.dt.float32, tag="sb")
                for i in range(2):
                    nc.tensor.matmul(sb[0:U, i, :], lhsT=Ib[0:U, :], rhs=qdup,
                                     start=True, stop=False, skip_group_check=True)
                for i in range(2):
                    nc.tensor.matmul(sb[0:U, i, :], lhsT=A2b[0:U, :],
                                     rhs=st_ap(t0 + i),
                                     start=False, stop=True, skip_group_check=True)
                if k >= 1:
                    emit_vr(k - 1)

                g = gp2.tile([U, 2, NB], mybir.dt.bfloat16, tag="g")
                nc.scalar.activation(g, sb[0:U, :, :], AF.Sigmoid)
                gs[k] = g

                if k >= 1:
                    emit_tmp(k - 1)

            emit_vr(NK - 1)
            emit_tmp(NK - 1)
            emit_acc(NK - 2, start=False, stop=False)
            emit_acc(NK - 1, start=False, stop=True)

            osb = gp2.tile([U, NB], mybir.dt.float32, tag="osb")
            nc.vector.tensor_copy(osb, acc[0:U, :])
            nc.sync.dma_start(out=outraw[:, :], in_=osb)

    nc.compile()
    return nc


def _prep_weights(kernel_w, rec_kernel, bias):
    b0, b1 = bias[0], bias[1]
    w = {}
    Kz = np.zeros((102, U), np.float32)
    Kz[:E] = -kernel_w[:, :U]
    Kz[100, :] = -40.0
    Kz[101, :] = -(b0[:U] + b1[:U])
    Kr = np.zeros((102, U), np.float32)
    Kr[:E] = kernel_w[:, U:2 * U]
    Kr[101, :] = b0[U:2 * U] + b1[U:2 * U]
    Kh = np.zeros((102, U), np.float32)
    Kh[:E] = kernel_w[:, 2 * U:]
    Kh[101, :] = b0[2 * U:]
    w["wKz"], w["wKr"], w["wKh"] = Kz, Kr, Kh
    Rz = np.zeros((101, U), np.float32)
    Rz[:U] = -rec_kernel[:, :U]
    Rr = np.zeros((101, U), np.float32)
    Rr[:U] = rec_kernel[:, U:2 * U]
    Rh = np.zeros((101, U), np.float32)
    Rh[:U] = rec_kernel[:, 2 * U:]
    Rh[100, :] = b1[2 * U:]
    w["wRz"], w["wRr"], w["wRh"] = Rz, Rr, Rh
    return {k: v.astype(bf16) for k, v in w.items()}


def _build_in_maps(inputs):
    session_hidden = np.asarray(inputs["session_hidden"], np.float32)
    mask = np.asarray(inputs["mask"], np.float32)
    kernel_w = np.asarray(inputs["kernel"], np.float32)
    rec_kernel = np.asarray(inputs["rec_kernel"], np.float32)
    bias = np.asarray(inputs["bias"], np.float32)
    A1_w = np.asarray(inputs["A1_w"], np.float32)
    A2_w = np.asarray(inputs["A2_w"], np.float32)
    v = np.asarray(inputs["v"], np.float32)

    w = _prep_weights(kernel_w, rec_kernel, bias)
    w["wA1"] = A1_w.astype(bf16)
    w["wA2"] = A2_w.astype(bf16)
    w["wVr"] = np.broadcast_to(v[0][:, None], (U, U)).astype(bf16).copy()
    w["wI"] = np.eye(U, dtype=np.float32).astype(bf16)
    w["wOnes"] = np.ones((1, W, NB), np.float32).astype(bf16)

    # xaug: [102, T, 512] per core; rows 0:100 = x^T, 100 = 1-m, 101 = 1
    x = session_hidden.reshape(NCORES, PERCORE, T, E)
    m = mask.reshape(NCORES, PERCORE, T)
    in_maps = []
    for k in range(NCORES):
        xa = np.zeros((102, T, NB), np.float32)
        xa[:E] = x[k].transpose(2, 1, 0)       # [E, T, 512]
        xa[100] = 1.0 - m[k].T                 # [T, 512]
        xa[101] = 1.0
        im = dict(w)
        im["xaug"] = xa.astype(bf16)
        in_maps.append(im)
    return in_maps


def kernel(session_hidden, mask, kernel, rec_kernel, bias, A1_w, A2_w, v):
    mask = np.asarray(mask, np.float32)
    A1_w = np.asarray(A1_w, np.float32)
    A2_w = np.asarray(A2_w, np.float32)
    v = np.asarray(v, np.float32)

    if "nc" not in _CACHE:
        _CACHE["nc"] = _build()
    nc = _CACHE["nc"]

    in_maps = _build_in_maps(dict(session_hidden=session_hidden, mask=mask,
                                  kernel=kernel, rec_kernel=rec_kernel, bias=bias,
                                  A1_w=A1_w, A2_w=A2_w, v=v))

    res = bass_utils.run_bass_kernel_spmd(nc, in_maps, core_ids=list(range(NCORES)))

    out_raw = np.zeros((B, U), np.float32)
    last = np.zeros((B, U), np.float32)
    for k in range(NCORES):
        r = res.results[k]
        sl = slice(k * PERCORE, (k + 1) * PERCORE)
        out_raw[sl] = np.asarray(r["outraw"]).T.astype(np.float32)
        last[sl] = np.asarray(r["lastout"]).T.astype(np.float32)

    # host correction for masked timesteps (device used last@A1 term for ALL t)
    lengths = mask.sum(1)
    sl_ = last @ A2_w
    c_ = last @ A1_w
    sig = lambda a: 1.0 / (1.0 + np.exp(-a))
    a_corr = (sig(sl_ + c_) - sig(sl_)) @ v[0]
    out = out_raw - (T - lengths)[:, None] * a_corr[:, None] * last
    return out.astype(np.float32)
